# revision 38
# baseline (speedup 1.0000x reference)
"""AGNNConv distributed Trainium2 kernel (8 NeuronCores).

Strategy:
  - Destination nodes are range-partitioned across the 8 cores (12500 each),
    so segment-softmax and aggregation are fully core-local (no collectives).
  - Per core, edges are bucketed by (src-table-chunk q, dst-tile t) where a
    dst-tile is 128 consecutive local destination nodes.  Buckets are padded
    to multiples of 128 "edge slots"; the slot->bucket structure is shared
    across cores (max bucket size over cores) so one SPMD graph serves all.
  - Per-edge source rows are fetched with gpsimd.dma_gather (int16 indices,
    4 table chunks of 25088 rows each).
  - Per-edge destination rows are produced on-chip: a one-hot matrix M
    (edge-slot x dst-slot, built with a tensor_scalar is_equal against an
    iota row) is transposed on TensorE and used as lhsT to select rows of a
    locally prenormalized dst tile.  The same M is the lhsT of the
    segment-sum matmul that accumulates [weighted-feature | exp] columns
    into PSUM per dst-tile.
  - Softmax uses no max-subtraction: cos in [-1,1] and beta*cos/TEMP is
    bounded, and softmax is shift-invariant, so exp(e)/sum(exp(e)) equals
    the reference exactly.
"""

import sys
import os
import numpy as np

for _p in ('/opt/trn_rl_repo',):
    if _p not in sys.path and os.path.isdir(_p):
        sys.path.insert(0, _p)

from concourse import bass, bacc, mybir
import concourse.tile as tile
from concourse.bass_utils import run_bass_kernel_spmd
from concourse.masks import make_identity

P = 128
EPS = 1e-12
TEMP = 1.0

last_exec_ns = None


def _wrap16(arr, reps=8):
    # dma_gather index layout: element i at [i % 16, i // 16], replicated to
    # all 8 groups of 16 partitions.
    w = arr.reshape(-1, 16).T
    return np.ascontiguousarray(np.tile(w, (reps, 1)))


def _assign_nodes(src, dst, n_nodes, n_cores, tiles, chunk, nchunks):
    """Degree-balanced node -> (core, tile, slot) assignment.

    Greedy batched deal: nodes sorted by in-degree descending; each round
    assigns one node to every bin, pairing heavy nodes with the bins whose
    per-src-chunk load vector is lightest.  Keeps every (q, tile, core)
    bucket under the next 128-chunk boundary so the shared dma_gather
    structure stays minimal.
    """
    nbins = n_cores * tiles
    degq = np.zeros((n_nodes, nchunks), dtype=np.int64)
    np.add.at(degq, (np.asarray(dst, dtype=np.int64),
                     np.asarray(src, dtype=np.int64) // chunk), 1)
    deg = degq.sum(1)
    order = np.argsort(-deg, kind='stable')
    loads = np.zeros((nbins, nchunks), dtype=np.int64)
    node_bin = np.empty(n_nodes, dtype=np.int64)
    bin_n = np.zeros(nbins, dtype=np.int64)
    step = max(1, nbins // 2)
    b0 = 0
    while b0 < n_nodes:
        open_bins = np.nonzero(bin_n < P)[0]
        k = min(step, len(open_bins), n_nodes - b0)
        batch = order[b0:b0 + k]
        metric = loads[open_bins].max(axis=1)
        sel = open_bins[np.argsort(metric, kind='stable')[:k]]
        node_bin[batch] = sel
        loads[sel] += degq[batch]
        bin_n[sel] += 1
        b0 += k
    # slot = rank within bin
    ord2 = np.argsort(node_bin, kind='stable')
    counts = np.bincount(node_bin, minlength=nbins)
    assert counts.max() <= P, counts.max()
    start = np.concatenate([[0], np.cumsum(counts)[:-1]])
    slot = np.empty(n_nodes, dtype=np.int64)
    slot[ord2] = np.arange(n_nodes) - start[node_bin[ord2]]
    node_core = node_bin // tiles
    node_tile = node_bin % tiles
    return node_core, node_tile, slot


def _host_structure(src, dst, n_nodes, n_cores, nloc, tiles, chunk, nchunks):
    """Bucket edges per core by (q, t); build shared structure + per-core
    padded index/slot streams."""
    src = np.asarray(src, dtype=np.int64)
    dst = np.asarray(dst, dtype=np.int64)
    node_core, node_tile, node_slot = _assign_nodes(src, dst, n_nodes, n_cores, tiles, chunk, nchunks)
    core = node_core[dst]
    nbuckets = nchunks * tiles

    per_core = []
    counts = np.zeros((n_cores, nbuckets), dtype=np.int64)
    for c in range(n_cores):
        sel = core == c
        s_c = src[sel]
        d_c = dst[sel]
        t_c = node_tile[d_c]
        slot_c = node_slot[d_c]
        q_c = s_c // chunk
        key = q_c * tiles + t_c
        order = np.argsort(key, kind='stable')
        s_c, slot_c, key = s_c[order], slot_c[order], key[order]
        counts[c] = np.bincount(key, minlength=nbuckets)
        per_core.append((s_c, slot_c, key))

    # shared structure: per-bucket chunk count = ceil(max over cores / 128)
    bucket_chunks = (counts.max(axis=0) + P - 1) // P  # [nbuckets]
    bucket_slots = bucket_chunks * P
    bucket_off = np.zeros(nbuckets + 1, dtype=np.int64)
    np.cumsum(bucket_slots, out=bucket_off[1:])
    s_total = int(bucket_off[-1])

    gidx_streams = []
    slot_streams = []
    for c in range(n_cores):
        s_c, slot_c, key = per_core[c]
        gidx = np.zeros(s_total, dtype=np.int16)
        slots = np.full(s_total, 255.0, dtype=np.float32)
        # position of edge i (sorted by key): bucket_off[key] + rank in bucket
        cum = np.cumsum(np.bincount(key, minlength=nbuckets))
        start_in_sorted = np.concatenate([[0], cum[:-1]])
        rank = np.arange(len(key)) - start_in_sorted[key]
        pos = bucket_off[key] + rank
        q_c = s_c // chunk
        gidx[pos] = (s_c - q_c * chunk).astype(np.int16)
        slots[pos] = slot_c.astype(np.float32)
        gidx_streams.append(_wrap16(gidx))
        import ml_dtypes
        slot_streams.append(np.ascontiguousarray(slots.reshape(-1, P).T.astype(ml_dtypes.bfloat16)))

    return (bucket_chunks, bucket_off, s_total, gidx_streams, slot_streams,
            (node_core, node_tile, node_slot))


def _build_graph(cfg, bucket_chunks, bucket_off, s_total):
    n_pad = cfg['n_pad']
    d = cfg['d']
    tiles = cfg['tiles']
    chunk = cfg['chunk']
    nchunks = cfg['nchunks']
    nloc_pad = tiles * P
    SUB = 16         # chunks per DVE sub-block
    GBLK = 16        # max chunks per dma_gather block

    f32 = mybir.dt.float32
    bf16 = mybir.dt.bfloat16
    nc = bacc.Bacc("TRN2", target_bir_lowering=False, debug=False, num_devices=8)

    feat_ext = nc.declare_dram_parameter("feat", [n_pad, d], f32, isOutput=False)
    locfeat_ext = nc.declare_dram_parameter("locfeat", [nloc_pad, d], f32, isOutput=False)
    beta_ext = nc.declare_dram_parameter("beta128", [P, 1], f32, isOutput=False)
    iota_ext = nc.declare_dram_parameter("iota128", [P, P], bf16, isOutput=False)
    gidx_ext = nc.declare_dram_parameter("gidx", [P, s_total // 16], mybir.dt.int16, isOutput=False)
    slot_ext = nc.declare_dram_parameter("slotw", [P, s_total // P], bf16, isOutput=False)
    out_ext = nc.declare_dram_parameter("out", [nloc_pad, d], f32, isOutput=True)

    eq = mybir.AluOpType.is_equal
    mul = mybir.AluOpType.mult
    add = mybir.AluOpType.add
    AF = mybir.ActivationFunctionType
    AX = mybir.AxisListType

    with tile.TileContext(nc) as tc:
        with (
            tc.tile_pool(name="const", bufs=1) as cpool,
            tc.tile_pool(name="tsc", bufs=1) as tscpool,
            tc.tile_pool(name="acc", bufs=1) as accpool,
            tc.tile_pool(name="tbuild", bufs=3) as tbpool,
            tc.tile_pool(name="small", bufs=6) as smpool,
            tc.tile_pool(name="gath", bufs=4) as gpool,
            tc.tile_pool(name="mpool", bufs=3) as mpool,
            tc.tile_pool(name="mts", bufs=4) as mtspool,
            tc.tile_pool(name="prod", bufs=3) as prodpool,
            tc.tile_pool(name="xw", bufs=3) as xwpool,
            tc.tile_pool(name="idx", bufs=4) as idxpool,
            tc.tile_pool(name="psA", bufs=2, space="PSUM") as psA,      # M^T
            tc.tile_pool(name="psB", bufs=3, space="PSUM") as psB,      # D_edge
            tc.tile_pool(name="psC", bufs=3, space="PSUM") as psC,      # acc
        ):
            iota_t = cpool.tile([P, P], bf16)
            nc.sync.dma_start(out=iota_t[:], in_=iota_ext[:])
            beta_t = cpool.tile([P, 1], f32)
            nc.sync.dma_start(out=beta_t[:], in_=beta_ext[:])
            ident = cpool.tile([P, P], bf16)
            make_identity(nc, ident[:])

            tsc = tscpool.tile([P, tiles, d], bf16, tag="tscb")  # prenormalized dst rows
            tscf = tscpool.tile([P, tiles, d], f32, tag="tscf")
            accum = accpool.tile([P, tiles, d + 1], f32)
            nc.vector.memset(accum[:], 0.0)

            # ---- phase 1: build T_scaled = feat_loc / max(||feat_loc||, EPS)
            for t in range(tiles):
                traw = tbpool.tile([P, d], f32, tag="traw")
                nc.sync.dma_start(out=traw[:], in_=locfeat_ext[t * P:(t + 1) * P, :])
                sq = tbpool.tile([P, d], f32, tag="sq")
                ld2 = smpool.tile([P, 1], f32, tag="ld2")
                nc.scalar.activation(sq[:], traw[:], AF.Square, accum_out=ld2[:])
                ld = smpool.tile([P, 1], f32, tag="ld")
                nc.scalar.activation(ld[:], ld2[:], AF.Sqrt)
                ldc = smpool.tile([P, 1], f32, tag="ldc")
                nc.vector.tensor_scalar_max(out=ldc[:], in0=ld[:], scalar1=EPS)
                ild = smpool.tile([P, 1], f32, tag="ild")
                nc.vector.reciprocal(ild[:], ldc[:])
                nc.vector.tensor_scalar_mul(out=tscf[:, t, :], in0=traw[:], scalar1=ild[:])
            nc.vector.tensor_copy(out=tsc[:], in_=tscf[:])

            # ---- phase 2: edge stream
            # chunk -> bucket map from shared structure
            nbuckets = nchunks * tiles
            for q in range(nchunks):
                q_first_bucket = q * tiles
                q_start = int(bucket_off[q_first_bucket])
                q_end = int(bucket_off[(q + 1) * tiles])
                q_nch = (q_end - q_start) // P
                # chunk-global -> (bucket, first?, last?) within this q
                chunk_bucket = []
                for t in range(tiles):
                    b = q_first_bucket + t
                    for j in range(int(bucket_chunks[b])):
                        chunk_bucket.append((b, j == 0, j == int(bucket_chunks[b]) - 1))
                assert len(chunk_bucket) == q_nch

                acc_ps = None
                for blk0 in range(0, q_nch, GBLK):
                    nch = min(GBLK, q_nch - blk0)
                    base_slot = q_start + blk0 * P

                    idx_t = idxpool.tile([P, GBLK * 8], mybir.dt.int16, tag="idx")
                    nc.sync.dma_start(
                        out=idx_t[:, :nch * 8],
                        in_=gidx_ext[:, base_slot // 16:(base_slot + nch * P) // 16])
                    slot_t = idxpool.tile([P, GBLK], bf16, tag="slot")
                    nc.sync.dma_start(
                        out=slot_t[:, :nch],
                        in_=slot_ext[:, base_slot // P:base_slot // P + nch])

                    g = gpool.tile([P, GBLK, d], f32, tag="g")
                    nc.gpsimd.dma_gather(
                        out_ap=g[:, :nch, :],
                        in_ap=feat_ext[q * chunk:(q + 1) * chunk, :],
                        idxs_ap=idx_t[:, :nch * 8],
                        num_idxs=nch * P,
                        num_idxs_reg=nch * P,
                        elem_size=d,
                        single_packet=False,
                    )

                    m_t = mpool.tile([P, GBLK, P], bf16, tag="m")
                    nc.vector.tensor_tensor(
                        out=m_t[:, :nch, :],
                        in0=slot_t[:, :nch, None].to_broadcast([P, nch, P]),
                        in1=iota_t[:, None, :].to_broadcast([P, nch, P]),
                        op=eq)

                    # M^T via TensorE transposes (groups of 4) + ACT evacuation,
                    # then per-edge dst rows via M @ T_scaled into half-block
                    # PSUM tiles (1 bank each).
                    HALF = 8
                    dps_halves = []
                    for g0 in range(0, nch, 4):
                        ng = min(4, nch - g0)
                        mtp = psA.tile([P, 4, P], bf16, tag="mtp")
                        for j in range(g0, g0 + ng):
                            nc.tensor.transpose(
                                mtp[:, j - g0, :], m_t[:, j, :], ident[:])
                        mts = mtspool.tile([P, 4, P], bf16, tag="mts")
                        nc.scalar.activation(
                            mts[:, :ng, :], mtp[:, :ng, :], AF.Copy)
                        if g0 % HALF == 0:
                            dps = psB.tile([P, HALF, d], f32, tag="dps")
                            dps_halves.append(dps)
                        for j in range(g0, g0 + ng):
                            cgl = blk0 + j
                            b, _, _ = chunk_bucket[cgl]
                            t = b - q_first_bucket
                            nc.tensor.matmul(
                                dps[:, j % HALF, :], lhsT=mts[:, j - g0, :],
                                rhs=tsc[:, t, :], start=True, stop=True)

                    S_blk = g[:, :nch, :]
                    ssq = prodpool.tile([P, GBLK, d], f32, tag="ssq")
                    nc.scalar.activation(ssq[:, :nch, :], S_blk, AF.Square)
                    ls2 = smpool.tile([P, GBLK], f32, tag="ls2")
                    nc.vector.tensor_reduce(
                        out=ls2[:, :nch], in_=ssq[:, :nch, :], axis=AX.X, op=add)
                    cosn = smpool.tile([P, GBLK], f32, tag="cosn")
                    for hi, dps in enumerate(dps_halves):
                        h0 = hi * HALF
                        nh = min(HALF, nch - h0)
                        sdp = prodpool.tile([P, HALF, d], f32, tag="sdp")
                        nc.vector.tensor_tensor(
                            out=sdp[:, :nh, :], in0=g[:, h0:h0 + nh, :],
                            in1=dps[:, :nh, :], op=mul)
                        nc.vector.tensor_reduce(
                            out=cosn[:, h0:h0 + nh], in_=sdp[:, :nh, :], axis=AX.X, op=add)
                    ls = smpool.tile([P, GBLK], f32, tag="ls")
                    nc.scalar.activation(ls[:, :nch], ls2[:, :nch], AF.Sqrt)
                    ils = smpool.tile([P, GBLK], f32, tag="ils")
                    nc.vector.reciprocal(ils[:, :nch], ls[:, :nch])
                    lg = smpool.tile([P, GBLK], f32, tag="lg")
                    nc.vector.tensor_tensor(
                        out=lg[:, :nch], in0=cosn[:, :nch], in1=ils[:, :nch], op=mul)
                    pt = smpool.tile([P, GBLK], f32, tag="pt")
                    nc.scalar.activation(
                        pt[:, :nch], lg[:, :nch], AF.Exp, scale=beta_t[:, 0:1])
                    xw = xwpool.tile([P, GBLK, d + 1], bf16, tag="xw")
                    nc.vector.tensor_tensor(
                        out=xw[:, :nch, 0:d], in0=S_blk,
                        in1=pt[:, :nch, None].to_broadcast([P, nch, d]), op=mul)
                    nc.vector.tensor_copy(out=xw[:, :nch, d], in_=pt[:, :nch])

                    # scatter matmuls
                    for j in range(nch):
                        cgl = blk0 + j
                        b, first, last = chunk_bucket[cgl]
                        t = b - q_first_bucket
                        if first:
                            acc_ps = psC.tile([P, d + 1], f32, tag="accps")
                        nc.tensor.matmul(
                            acc_ps[:], lhsT=m_t[:, j, :],
                            rhs=xw[:, j, :], start=first, stop=last)
                        if last:
                            nc.vector.tensor_tensor(
                                out=accum[:, t, :], in0=accum[:, t, :],
                                in1=acc_ps[:], op=add)

            # ---- phase 3: normalize + writeback
            for t in range(tiles):
                r = smpool.tile([P, 1], f32, tag="r")
                nc.vector.reciprocal(r[:], accum[:, t, d:d + 1])
                ostg = tbpool.tile([P, d], f32, tag="ostg")
                nc.vector.tensor_scalar_mul(out=ostg[:], in0=accum[:, t, 0:d], scalar1=r[:])
                nc.sync.dma_start(out=out_ext[t * P:(t + 1) * P, :], in_=ostg[:])

    nc.compile()
    return nc


def _run(feat, beta, src, dst, cfg):
    global last_exec_ns
    n = cfg['n']
    n_pad = cfg['n_pad']
    d = cfg['d']
    n_cores = cfg['n_cores']
    nloc = cfg['nloc']
    tiles = cfg['tiles']
    chunk = cfg['chunk']
    nchunks = cfg['nchunks']
    nloc_pad = tiles * P

    feat = np.ascontiguousarray(np.asarray(feat, dtype=np.float32))
    beta = np.asarray(beta, dtype=np.float32)

    (bucket_chunks, bucket_off, s_total, gidx_streams, slot_streams,
     (node_core, node_tile, node_slot)) = _host_structure(
        src, dst, n, n_cores, nloc, tiles, chunk, nchunks)

    nc = _build_graph(cfg, bucket_chunks, bucket_off, s_total)

    feat_pad = np.zeros((n_pad, d), dtype=np.float32)
    feat_pad[:n] = feat
    beta128 = np.full((P, 1), beta.reshape(-1)[0], dtype=np.float32)
    import ml_dtypes
    iota128 = np.broadcast_to(np.arange(P).astype(ml_dtypes.bfloat16), (P, P)).copy()

    node_pos = node_tile * P + node_slot  # local row within the owning core
    in_maps = []
    for c in range(n_cores):
        locfeat = np.zeros((nloc_pad, d), dtype=np.float32)
        mine = np.nonzero(node_core == c)[0]
        locfeat[node_pos[mine]] = feat[mine]
        in_maps.append({
            "feat": feat_pad,
            "locfeat": locfeat,
            "beta128": beta128,
            "iota128": iota128,
            "gidx": gidx_streams[c],
            "slotw": slot_streams[c],
        })

    res = run_bass_kernel_spmd(nc, in_maps, core_ids=list(range(n_cores)),
                               trace=cfg.get('trace', False))
    last_exec_ns = res.exec_time_ns

    out = np.empty((n, d), dtype=np.float32)
    for c in range(n_cores):
        mine = np.nonzero(node_core == c)[0]
        out[mine] = res.results[c]["out"][node_pos[mine]]
    return out


FULL_CFG = dict(n=100000, n_pad=100352, d=64, n_cores=8, nloc=12500,
                tiles=104, chunk=25088, nchunks=4)


def kernel(feat, beta, src, dst):
    return _run(feat, beta, src, dst, dict(FULL_CFG))


# revision 40
# speedup vs baseline: 1.1809x; 1.1809x over previous
"""AGNNConv distributed Trainium2 kernel (8 NeuronCores).

Strategy:
  - Destination nodes are range-partitioned across the 8 cores (12500 each),
    so segment-softmax and aggregation are fully core-local (no collectives).
  - Per core, edges are bucketed by (src-table-chunk q, dst-tile t) where a
    dst-tile is 128 consecutive local destination nodes.  Buckets are padded
    to multiples of 128 "edge slots"; the slot->bucket structure is shared
    across cores (max bucket size over cores) so one SPMD graph serves all.
  - Per-edge source rows are fetched with gpsimd.dma_gather (int16 indices,
    4 table chunks of 25088 rows each).
  - Per-edge destination rows are produced on-chip: a one-hot matrix M
    (edge-slot x dst-slot, built with a tensor_scalar is_equal against an
    iota row) is transposed on TensorE and used as lhsT to select rows of a
    locally prenormalized dst tile.  The same M is the lhsT of the
    segment-sum matmul that accumulates [weighted-feature | exp] columns
    into PSUM per dst-tile.
  - Softmax uses no max-subtraction: cos in [-1,1] and beta*cos/TEMP is
    bounded, and softmax is shift-invariant, so exp(e)/sum(exp(e)) equals
    the reference exactly.
"""

import sys
import os
import numpy as np

for _p in ('/opt/trn_rl_repo',):
    if _p not in sys.path and os.path.isdir(_p):
        sys.path.insert(0, _p)

from concourse import bass, bacc, mybir
import concourse.tile as tile
from concourse.bass_utils import run_bass_kernel_spmd
from concourse.masks import make_identity

P = 128
EPS = 1e-12
TEMP = 1.0

last_exec_ns = None


def _wrap16(arr, reps=8):
    # dma_gather index layout: element i at [i % 16, i // 16], replicated to
    # all 8 groups of 16 partitions.
    w = arr.reshape(-1, 16).T
    return np.ascontiguousarray(np.tile(w, (reps, 1)))


def _assign_nodes(src, dst, n_nodes, n_cores, tiles, chunk, nchunks):
    """Degree-balanced node -> (core, tile, slot) assignment.

    Greedy batched deal: nodes sorted by in-degree descending; each round
    assigns one node to every bin, pairing heavy nodes with the bins whose
    per-src-chunk load vector is lightest.  Keeps every (q, tile, core)
    bucket under the next 128-chunk boundary so the shared dma_gather
    structure stays minimal.
    """
    nbins = n_cores * tiles
    degq = np.zeros((n_nodes, nchunks), dtype=np.int64)
    np.add.at(degq, (np.asarray(dst, dtype=np.int64),
                     np.asarray(src, dtype=np.int64) // chunk), 1)
    deg = degq.sum(1)
    order = np.argsort(-deg, kind='stable')
    loads = np.zeros((nbins, nchunks), dtype=np.int64)
    node_bin = np.empty(n_nodes, dtype=np.int64)
    bin_n = np.zeros(nbins, dtype=np.int64)
    step = max(1, nbins // 2)
    b0 = 0
    while b0 < n_nodes:
        open_bins = np.nonzero(bin_n < P)[0]
        k = min(step, len(open_bins), n_nodes - b0)
        batch = order[b0:b0 + k]
        metric = loads[open_bins].max(axis=1)
        sel = open_bins[np.argsort(metric, kind='stable')[:k]]
        node_bin[batch] = sel
        loads[sel] += degq[batch]
        bin_n[sel] += 1
        b0 += k
    # slot = rank within bin
    ord2 = np.argsort(node_bin, kind='stable')
    counts = np.bincount(node_bin, minlength=nbins)
    assert counts.max() <= P, counts.max()
    start = np.concatenate([[0], np.cumsum(counts)[:-1]])
    slot = np.empty(n_nodes, dtype=np.int64)
    slot[ord2] = np.arange(n_nodes) - start[node_bin[ord2]]
    node_core = node_bin // tiles
    node_tile = node_bin % tiles
    return node_core, node_tile, slot


def _host_structure(src, dst, n_nodes, n_cores, nloc, tiles, chunk, nchunks):
    """Bucket edges per core by (q, t); build shared structure + per-core
    padded index/slot streams."""
    src = np.asarray(src, dtype=np.int64)
    dst = np.asarray(dst, dtype=np.int64)
    node_core, node_tile, node_slot = _assign_nodes(src, dst, n_nodes, n_cores, tiles, chunk, nchunks)
    core = node_core[dst]
    nbuckets = nchunks * tiles

    per_core = []
    counts = np.zeros((n_cores, nbuckets), dtype=np.int64)
    for c in range(n_cores):
        sel = core == c
        s_c = src[sel]
        d_c = dst[sel]
        t_c = node_tile[d_c]
        slot_c = node_slot[d_c]
        q_c = s_c // chunk
        key = q_c * tiles + t_c
        order = np.argsort(key, kind='stable')
        s_c, slot_c, key = s_c[order], slot_c[order], key[order]
        counts[c] = np.bincount(key, minlength=nbuckets)
        per_core.append((s_c, slot_c, key))

    # shared structure: per-bucket chunk count = ceil(max over cores / 128)
    bucket_chunks = (counts.max(axis=0) + P - 1) // P  # [nbuckets]
    bucket_slots = bucket_chunks * P
    bucket_off = np.zeros(nbuckets + 1, dtype=np.int64)
    np.cumsum(bucket_slots, out=bucket_off[1:])
    s_total = int(bucket_off[-1])

    gidx_streams = []
    slot_streams = []
    for c in range(n_cores):
        s_c, slot_c, key = per_core[c]
        gidx = np.zeros(s_total, dtype=np.int16)
        slots = np.full(s_total, 255.0, dtype=np.float32)
        # position of edge i (sorted by key): bucket_off[key] + rank in bucket
        cum = np.cumsum(np.bincount(key, minlength=nbuckets))
        start_in_sorted = np.concatenate([[0], cum[:-1]])
        rank = np.arange(len(key)) - start_in_sorted[key]
        pos = bucket_off[key] + rank
        q_c = s_c // chunk
        gidx[pos] = (s_c - q_c * chunk).astype(np.int16)
        slots[pos] = slot_c.astype(np.float32)
        gidx_streams.append(_wrap16(gidx))
        import ml_dtypes
        slot_streams.append(np.ascontiguousarray(slots.reshape(-1, P).T.astype(ml_dtypes.bfloat16)))

    return (bucket_chunks, bucket_off, s_total, gidx_streams, slot_streams,
            (node_core, node_tile, node_slot))


def _build_graph(cfg, bucket_chunks, bucket_off, s_total):
    n_pad = cfg['n_pad']
    d = cfg['d']
    tiles = cfg['tiles']
    chunk = cfg['chunk']
    nchunks = cfg['nchunks']
    nloc_pad = tiles * P
    SUB = 16         # chunks per DVE sub-block
    GBLK = 16        # max chunks per dma_gather block

    f32 = mybir.dt.float32
    bf16 = mybir.dt.bfloat16
    nc = bacc.Bacc("TRN2", target_bir_lowering=False, debug=False, num_devices=8)

    feat_ext = nc.declare_dram_parameter("feat", [n_pad, d], f32, isOutput=False)
    locfeat_ext = nc.declare_dram_parameter("locfeat", [nloc_pad, d], f32, isOutput=False)
    beta_ext = nc.declare_dram_parameter("beta128", [P, 1], f32, isOutput=False)
    iota_ext = nc.declare_dram_parameter("iota128", [P, P], bf16, isOutput=False)
    gidx_ext = nc.declare_dram_parameter("gidx", [P, s_total // 16], mybir.dt.int16, isOutput=False)
    slot_ext = nc.declare_dram_parameter("slotw", [P, s_total // P], bf16, isOutput=False)
    out_ext = nc.declare_dram_parameter("out", [nloc_pad, d], f32, isOutput=True)

    eq = mybir.AluOpType.is_equal
    mul = mybir.AluOpType.mult
    add = mybir.AluOpType.add
    AF = mybir.ActivationFunctionType
    AX = mybir.AxisListType

    with tile.TileContext(nc) as tc:
        with (
            tc.tile_pool(name="const", bufs=1) as cpool,
            tc.tile_pool(name="tsc", bufs=1) as tscpool,
            tc.tile_pool(name="acc", bufs=1) as accpool,
            tc.tile_pool(name="tbuild", bufs=3) as tbpool,
            tc.tile_pool(name="small", bufs=8) as smpool,
            tc.tile_pool(name="gath", bufs=8) as gpool,
            tc.tile_pool(name="mpool", bufs=5) as mpool,
            tc.tile_pool(name="mts", bufs=6) as mtspool,
            tc.tile_pool(name="prod", bufs=4) as prodpool,
            tc.tile_pool(name="xw", bufs=4) as xwpool,
            tc.tile_pool(name="idx", bufs=8) as idxpool,
            tc.tile_pool(name="psA", bufs=2, space="PSUM") as psA,      # M^T
            tc.tile_pool(name="psB", bufs=3, space="PSUM") as psB,      # D_edge
            tc.tile_pool(name="psC", bufs=3, space="PSUM") as psC,      # acc
        ):
            iota_t = cpool.tile([P, P], bf16)
            nc.sync.dma_start(out=iota_t[:], in_=iota_ext[:])
            beta_t = cpool.tile([P, 1], f32)
            nc.sync.dma_start(out=beta_t[:], in_=beta_ext[:])
            ident = cpool.tile([P, P], bf16)
            make_identity(nc, ident[:])

            tsc = tscpool.tile([P, tiles, d], bf16)      # prenormalized dst rows
            accum = accpool.tile([P, tiles, d + 1], f32)
            nc.vector.memset(accum[:], 0.0)

            # ---- phase 1: build T_scaled = feat_loc / max(||feat_loc||, EPS)
            for t in range(tiles):
                traw = tbpool.tile([P, d], f32, tag="traw")
                nc.sync.dma_start(out=traw[:], in_=locfeat_ext[t * P:(t + 1) * P, :])
                sq = tbpool.tile([P, d], f32, tag="sq")
                ld2 = smpool.tile([P, 1], f32, tag="ld2")
                nc.scalar.activation(sq[:], traw[:], AF.Square, accum_out=ld2[:])
                ld = smpool.tile([P, 1], f32, tag="ld")
                nc.scalar.activation(ld[:], ld2[:], AF.Sqrt)
                ldc = smpool.tile([P, 1], f32, tag="ldc")
                nc.vector.tensor_scalar_max(out=ldc[:], in0=ld[:], scalar1=EPS)
                ild = smpool.tile([P, 1], f32, tag="ild")
                nc.vector.reciprocal(ild[:], ldc[:])
                nc.vector.tensor_scalar_mul(out=tsc[:, t, :], in0=traw[:], scalar1=ild[:])

            # ---- phase 2: edge stream
            # chunk -> bucket map from shared structure
            nbuckets = nchunks * tiles
            for q in range(nchunks):
                q_first_bucket = q * tiles
                q_start = int(bucket_off[q_first_bucket])
                q_end = int(bucket_off[(q + 1) * tiles])
                q_nch = (q_end - q_start) // P
                # chunk-global -> (bucket, first?, last?) within this q
                chunk_bucket = []
                for t in range(tiles):
                    b = q_first_bucket + t
                    for j in range(int(bucket_chunks[b])):
                        chunk_bucket.append((b, j == 0, j == int(bucket_chunks[b]) - 1))
                assert len(chunk_bucket) == q_nch

                acc_ps = None
                for blk0 in range(0, q_nch, GBLK):
                    nch = min(GBLK, q_nch - blk0)
                    base_slot = q_start + blk0 * P

                    idx_t = idxpool.tile([P, GBLK * 8], mybir.dt.int16, tag="idx")
                    nc.sync.dma_start(
                        out=idx_t[:, :nch * 8],
                        in_=gidx_ext[:, base_slot // 16:(base_slot + nch * P) // 16])
                    slot_t = idxpool.tile([P, GBLK], bf16, tag="slot")
                    nc.sync.dma_start(
                        out=slot_t[:, :nch],
                        in_=slot_ext[:, base_slot // P:base_slot // P + nch])

                    g = gpool.tile([P, GBLK, d], f32, tag="g")
                    nc.gpsimd.dma_gather(
                        out_ap=g[:, :nch, :],
                        in_ap=feat_ext[q * chunk:(q + 1) * chunk, :],
                        idxs_ap=idx_t[:, :nch * 8],
                        num_idxs=nch * P,
                        num_idxs_reg=nch * P,
                        elem_size=d,
                        single_packet=False,
                    )

                    m_t = mpool.tile([P, GBLK, P], bf16, tag="m")
                    nc.vector.tensor_tensor(
                        out=m_t[:, :nch, :],
                        in0=slot_t[:, :nch, None].to_broadcast([P, nch, P]),
                        in1=iota_t[:, None, :].to_broadcast([P, nch, P]),
                        op=eq)

                    # M^T via TensorE transposes (groups of 4) + ACT evacuation,
                    # then per-edge dst rows via M @ T_scaled into half-block
                    # PSUM tiles (1 bank each).
                    HALF = 8
                    dps_halves = []
                    for g0 in range(0, nch, 4):
                        ng = min(4, nch - g0)
                        mtp = psA.tile([P, 4, P], bf16, tag="mtp")
                        for j in range(g0, g0 + ng):
                            nc.tensor.transpose(
                                mtp[:, j - g0, :], m_t[:, j, :], ident[:])
                        mts = mtspool.tile([P, 4, P], bf16, tag="mts")
                        nc.scalar.activation(
                            mts[:, :ng, :], mtp[:, :ng, :], AF.Copy)
                        if g0 % HALF == 0:
                            dps = psB.tile([P, HALF, d], f32, tag="dps")
                            dps_halves.append(dps)
                        for j in range(g0, g0 + ng):
                            cgl = blk0 + j
                            b, _, _ = chunk_bucket[cgl]
                            t = b - q_first_bucket
                            nc.tensor.matmul(
                                dps[:, j % HALF, :], lhsT=mts[:, j - g0, :],
                                rhs=tsc[:, t, :], start=True, stop=True)

                    S_blk = g[:, :nch, :]
                    ssq = prodpool.tile([P, GBLK, d], f32, tag="ssq")
                    nc.vector.tensor_tensor(out=ssq[:, :nch, :], in0=S_blk, in1=S_blk, op=mul)
                    ls2 = smpool.tile([P, GBLK], f32, tag="ls2")
                    nc.vector.tensor_reduce(
                        out=ls2[:, :nch], in_=ssq[:, :nch, :], axis=AX.X, op=add)
                    cosn = smpool.tile([P, GBLK], f32, tag="cosn")
                    for hi, dps in enumerate(dps_halves):
                        h0 = hi * HALF
                        nh = min(HALF, nch - h0)
                        sdp = prodpool.tile([P, HALF, d], f32, tag="sdp")
                        nc.vector.tensor_tensor(
                            out=sdp[:, :nh, :], in0=g[:, h0:h0 + nh, :],
                            in1=dps[:, :nh, :], op=mul)
                        nc.vector.tensor_reduce(
                            out=cosn[:, h0:h0 + nh], in_=sdp[:, :nh, :], axis=AX.X, op=add)
                    ls = smpool.tile([P, GBLK], f32, tag="ls")
                    nc.scalar.activation(ls[:, :nch], ls2[:, :nch], AF.Sqrt)
                    ils = smpool.tile([P, GBLK], f32, tag="ils")
                    nc.vector.reciprocal(ils[:, :nch], ls[:, :nch])
                    lg = smpool.tile([P, GBLK], f32, tag="lg")
                    nc.vector.tensor_tensor(
                        out=lg[:, :nch], in0=cosn[:, :nch], in1=ils[:, :nch], op=mul)
                    pt = smpool.tile([P, GBLK], f32, tag="pt")
                    nc.scalar.activation(
                        pt[:, :nch], lg[:, :nch], AF.Exp, scale=beta_t[:, 0:1])
                    xw = xwpool.tile([P, GBLK, d + 1], bf16, tag="xw")
                    nc.vector.tensor_tensor(
                        out=xw[:, :nch, 0:d], in0=S_blk,
                        in1=pt[:, :nch, None].to_broadcast([P, nch, d]), op=mul)
                    nc.vector.tensor_copy(out=xw[:, :nch, d], in_=pt[:, :nch])

                    # scatter matmuls
                    for j in range(nch):
                        cgl = blk0 + j
                        b, first, last = chunk_bucket[cgl]
                        t = b - q_first_bucket
                        if first:
                            acc_ps = psC.tile([P, d + 1], f32, tag="accps")
                        nc.tensor.matmul(
                            acc_ps[:], lhsT=m_t[:, j, :],
                            rhs=xw[:, j, :], start=first, stop=last)
                        if last:
                            nc.vector.tensor_tensor(
                                out=accum[:, t, :], in0=accum[:, t, :],
                                in1=acc_ps[:], op=add)

            # ---- phase 3: normalize + writeback
            for t in range(tiles):
                r = smpool.tile([P, 1], f32, tag="r")
                nc.vector.reciprocal(r[:], accum[:, t, d:d + 1])
                ostg = tbpool.tile([P, d], f32, tag="ostg")
                nc.vector.tensor_scalar_mul(out=ostg[:], in0=accum[:, t, 0:d], scalar1=r[:])
                nc.sync.dma_start(out=out_ext[t * P:(t + 1) * P, :], in_=ostg[:])

    nc.compile()
    return nc


def _run(feat, beta, src, dst, cfg):
    global last_exec_ns
    n = cfg['n']
    n_pad = cfg['n_pad']
    d = cfg['d']
    n_cores = cfg['n_cores']
    nloc = cfg['nloc']
    tiles = cfg['tiles']
    chunk = cfg['chunk']
    nchunks = cfg['nchunks']
    nloc_pad = tiles * P

    feat = np.ascontiguousarray(np.asarray(feat, dtype=np.float32))
    beta = np.asarray(beta, dtype=np.float32)

    (bucket_chunks, bucket_off, s_total, gidx_streams, slot_streams,
     (node_core, node_tile, node_slot)) = _host_structure(
        src, dst, n, n_cores, nloc, tiles, chunk, nchunks)

    nc = _build_graph(cfg, bucket_chunks, bucket_off, s_total)

    feat_pad = np.zeros((n_pad, d), dtype=np.float32)
    feat_pad[:n] = feat
    beta128 = np.full((P, 1), beta.reshape(-1)[0], dtype=np.float32)
    import ml_dtypes
    iota128 = np.broadcast_to(np.arange(P).astype(ml_dtypes.bfloat16), (P, P)).copy()

    node_pos = node_tile * P + node_slot  # local row within the owning core
    in_maps = []
    for c in range(n_cores):
        locfeat = np.zeros((nloc_pad, d), dtype=np.float32)
        mine = np.nonzero(node_core == c)[0]
        locfeat[node_pos[mine]] = feat[mine]
        in_maps.append({
            "feat": feat_pad,
            "locfeat": locfeat,
            "beta128": beta128,
            "iota128": iota128,
            "gidx": gidx_streams[c],
            "slotw": slot_streams[c],
        })

    res = run_bass_kernel_spmd(nc, in_maps, core_ids=list(range(n_cores)),
                               trace=cfg.get('trace', False))
    last_exec_ns = res.exec_time_ns

    out = np.empty((n, d), dtype=np.float32)
    for c in range(n_cores):
        mine = np.nonzero(node_core == c)[0]
        out[mine] = res.results[c]["out"][node_pos[mine]]
    return out


FULL_CFG = dict(n=100000, n_pad=100352, d=64, n_cores=8, nloc=12500,
                tiles=104, chunk=25088, nchunks=4)


def kernel(feat, beta, src, dst):
    return _run(feat, beta, src, dst, dict(FULL_CFG))


# revision 42
# speedup vs baseline: 1.3047x; 1.1048x over previous
"""AGNNConv distributed Trainium2 kernel (8 NeuronCores).

Strategy:
  - Destination nodes are range-partitioned across the 8 cores (12500 each),
    so segment-softmax and aggregation are fully core-local (no collectives).
  - Per core, edges are bucketed by (src-table-chunk q, dst-tile t) where a
    dst-tile is 128 consecutive local destination nodes.  Buckets are padded
    to multiples of 128 "edge slots"; the slot->bucket structure is shared
    across cores (max bucket size over cores) so one SPMD graph serves all.
  - Per-edge source rows are fetched with gpsimd.dma_gather (int16 indices,
    4 table chunks of 25088 rows each).
  - Per-edge destination rows are produced on-chip: a one-hot matrix M
    (edge-slot x dst-slot, built with a tensor_scalar is_equal against an
    iota row) is transposed on TensorE and used as lhsT to select rows of a
    locally prenormalized dst tile.  The same M is the lhsT of the
    segment-sum matmul that accumulates [weighted-feature | exp] columns
    into PSUM per dst-tile.
  - Softmax uses no max-subtraction: cos in [-1,1] and beta*cos/TEMP is
    bounded, and softmax is shift-invariant, so exp(e)/sum(exp(e)) equals
    the reference exactly.
"""

import sys
import os
import numpy as np

for _p in ('/opt/trn_rl_repo',):
    if _p not in sys.path and os.path.isdir(_p):
        sys.path.insert(0, _p)

from concourse import bass, bacc, mybir
import concourse.tile as tile
from concourse.bass_utils import run_bass_kernel_spmd
from concourse.masks import make_identity

P = 128
EPS = 1e-12
TEMP = 1.0

last_exec_ns = None


def _wrap16(arr, reps=8):
    # dma_gather index layout: element i at [i % 16, i // 16], replicated to
    # all 8 groups of 16 partitions.
    w = arr.reshape(-1, 16).T
    return np.ascontiguousarray(np.tile(w, (reps, 1)))


def _assign_nodes(src, dst, n_nodes, n_cores, tiles, chunk, nchunks):
    """Degree-balanced node -> (core, tile, slot) assignment.

    Greedy batched deal: nodes sorted by in-degree descending; each round
    assigns one node to every bin, pairing heavy nodes with the bins whose
    per-src-chunk load vector is lightest.  Keeps every (q, tile, core)
    bucket under the next 128-chunk boundary so the shared dma_gather
    structure stays minimal.
    """
    nbins = n_cores * tiles
    degq = np.zeros((n_nodes, nchunks), dtype=np.int64)
    np.add.at(degq, (np.asarray(dst, dtype=np.int64),
                     np.asarray(src, dtype=np.int64) // chunk), 1)
    deg = degq.sum(1)
    order = np.argsort(-deg, kind='stable')
    loads = np.zeros((nbins, nchunks), dtype=np.int64)
    node_bin = np.empty(n_nodes, dtype=np.int64)
    bin_n = np.zeros(nbins, dtype=np.int64)
    step = max(1, nbins // 2)
    b0 = 0
    while b0 < n_nodes:
        open_bins = np.nonzero(bin_n < P)[0]
        k = min(step, len(open_bins), n_nodes - b0)
        batch = order[b0:b0 + k]
        metric = loads[open_bins].max(axis=1)
        sel = open_bins[np.argsort(metric, kind='stable')[:k]]
        node_bin[batch] = sel
        loads[sel] += degq[batch]
        bin_n[sel] += 1
        b0 += k
    # slot = rank within bin
    ord2 = np.argsort(node_bin, kind='stable')
    counts = np.bincount(node_bin, minlength=nbins)
    assert counts.max() <= P, counts.max()
    start = np.concatenate([[0], np.cumsum(counts)[:-1]])
    slot = np.empty(n_nodes, dtype=np.int64)
    slot[ord2] = np.arange(n_nodes) - start[node_bin[ord2]]
    node_core = node_bin // tiles
    node_tile = node_bin % tiles
    return node_core, node_tile, slot


def _host_structure(src, dst, n_nodes, n_cores, nloc, tiles, chunk, nchunks):
    """Bucket edges per core by (q, t); build shared structure + per-core
    padded index/slot streams."""
    src = np.asarray(src, dtype=np.int64)
    dst = np.asarray(dst, dtype=np.int64)
    node_core, node_tile, node_slot = _assign_nodes(src, dst, n_nodes, n_cores, tiles, chunk, nchunks)
    core = node_core[dst]
    nbuckets = nchunks * tiles

    per_core = []
    counts = np.zeros((n_cores, nbuckets), dtype=np.int64)
    for c in range(n_cores):
        sel = core == c
        s_c = src[sel]
        d_c = dst[sel]
        t_c = node_tile[d_c]
        slot_c = node_slot[d_c]
        q_c = s_c // chunk
        key = q_c * tiles + t_c
        order = np.argsort(key, kind='stable')
        s_c, slot_c, key = s_c[order], slot_c[order], key[order]
        counts[c] = np.bincount(key, minlength=nbuckets)
        per_core.append((s_c, slot_c, key))

    # shared structure: per-bucket chunk count = ceil(max over cores / 128)
    bucket_chunks = (counts.max(axis=0) + P - 1) // P  # [nbuckets]
    bucket_slots = bucket_chunks * P
    bucket_off = np.zeros(nbuckets + 1, dtype=np.int64)
    np.cumsum(bucket_slots, out=bucket_off[1:])
    s_total = int(bucket_off[-1])

    gidx_streams = []
    slot_streams = []
    for c in range(n_cores):
        s_c, slot_c, key = per_core[c]
        gidx = np.zeros(s_total, dtype=np.int16)
        slots = np.full(s_total, 255.0, dtype=np.float32)
        # position of edge i (sorted by key): bucket_off[key] + rank in bucket
        cum = np.cumsum(np.bincount(key, minlength=nbuckets))
        start_in_sorted = np.concatenate([[0], cum[:-1]])
        rank = np.arange(len(key)) - start_in_sorted[key]
        pos = bucket_off[key] + rank
        q_c = s_c // chunk
        gidx[pos] = (s_c - q_c * chunk).astype(np.int16)
        slots[pos] = slot_c.astype(np.float32)
        gidx_streams.append(_wrap16(gidx))
        import ml_dtypes
        slot_streams.append(np.ascontiguousarray(slots.reshape(-1, P).T.astype(ml_dtypes.bfloat16)))

    return (bucket_chunks, bucket_off, s_total, gidx_streams, slot_streams,
            (node_core, node_tile, node_slot))


def _build_graph(cfg, bucket_chunks, bucket_off, s_total):
    n_pad = cfg['n_pad']
    d = cfg['d']
    tiles = cfg['tiles']
    chunk = cfg['chunk']
    nchunks = cfg['nchunks']
    nloc_pad = tiles * P
    SUB = 16         # chunks per DVE sub-block
    GBLK = 16        # max chunks per dma_gather block

    f32 = mybir.dt.float32
    bf16 = mybir.dt.bfloat16
    nc = bacc.Bacc("TRN2", target_bir_lowering=False, debug=False, num_devices=8)

    feat_ext = nc.declare_dram_parameter("feat", [n_pad, d], f32, isOutput=False)
    locfeat_ext = nc.declare_dram_parameter("locfeat", [nloc_pad, d], f32, isOutput=False)
    beta_ext = nc.declare_dram_parameter("beta128", [P, 1], f32, isOutput=False)
    iota_ext = nc.declare_dram_parameter("iota128", [P, P], bf16, isOutput=False)
    gidx_ext = nc.declare_dram_parameter("gidx", [P, s_total // 16], mybir.dt.int16, isOutput=False)
    slot_ext = nc.declare_dram_parameter("slotw", [P, s_total // P], bf16, isOutput=False)
    out_ext = nc.declare_dram_parameter("out", [nloc_pad, d], f32, isOutput=True)

    eq = mybir.AluOpType.is_equal
    mul = mybir.AluOpType.mult
    add = mybir.AluOpType.add
    AF = mybir.ActivationFunctionType
    AX = mybir.AxisListType

    with tile.TileContext(nc) as tc:
        with (
            tc.tile_pool(name="const", bufs=1) as cpool,
            tc.tile_pool(name="tsc", bufs=1) as tscpool,
            tc.tile_pool(name="acc", bufs=1) as accpool,
            tc.tile_pool(name="tbuild", bufs=3) as tbpool,
            tc.tile_pool(name="small", bufs=6) as smpool,
            tc.tile_pool(name="gath", bufs=4) as gpool,
            tc.tile_pool(name="mpool", bufs=3) as mpool,
            tc.tile_pool(name="mts", bufs=4) as mtspool,
            tc.tile_pool(name="prod", bufs=3) as prodpool,
            tc.tile_pool(name="xw", bufs=3) as xwpool,
            tc.tile_pool(name="idx", bufs=4) as idxpool,
            tc.tile_pool(name="psA", bufs=2, space="PSUM") as psA,      # M^T
            tc.tile_pool(name="psB", bufs=3, space="PSUM") as psB,      # D_edge
            tc.tile_pool(name="psC", bufs=3, space="PSUM") as psC,      # acc
        ):
            iota_t = cpool.tile([P, P], bf16)
            nc.sync.dma_start(out=iota_t[:], in_=iota_ext[:])
            beta_t = cpool.tile([P, 1], f32)
            nc.sync.dma_start(out=beta_t[:], in_=beta_ext[:])
            ident = cpool.tile([P, P], bf16)
            make_identity(nc, ident[:])

            tsc = tscpool.tile([P, tiles, d], bf16)      # prenormalized dst rows
            accum = accpool.tile([P, tiles, d + 1], f32)
            nc.vector.memset(accum[:], 0.0)

            # ---- phase 1: build T_scaled = feat_loc / max(||feat_loc||, EPS)
            # Emitted lazily per tile right before first use so the edge
            # pipeline's consumers start immediately instead of waiting for
            # all 104 tile builds.
            built = set()

            def build_tile(t):
                built.add(t)
                traw = tbpool.tile([P, d], f32, tag="traw")
                nc.sync.dma_start(out=traw[:], in_=locfeat_ext[t * P:(t + 1) * P, :])
                sq = tbpool.tile([P, d], f32, tag="sq")
                ld2 = smpool.tile([P, 1], f32, tag="ld2")
                nc.scalar.activation(sq[:], traw[:], AF.Square, accum_out=ld2[:])
                ld = smpool.tile([P, 1], f32, tag="ld")
                nc.scalar.activation(ld[:], ld2[:], AF.Sqrt)
                ldc = smpool.tile([P, 1], f32, tag="ldc")
                nc.vector.tensor_scalar_max(out=ldc[:], in0=ld[:], scalar1=EPS)
                ild = smpool.tile([P, 1], f32, tag="ild")
                nc.vector.reciprocal(ild[:], ldc[:])
                nc.vector.tensor_scalar_mul(out=tsc[:, t, :], in0=traw[:], scalar1=ild[:])

            # ---- phase 2: edge stream
            # chunk -> bucket map from shared structure
            nbuckets = nchunks * tiles
            for q in range(nchunks):
                q_first_bucket = q * tiles
                q_start = int(bucket_off[q_first_bucket])
                q_end = int(bucket_off[(q + 1) * tiles])
                q_nch = (q_end - q_start) // P
                # chunk-global -> (bucket, first?, last?) within this q
                chunk_bucket = []
                for t in range(tiles):
                    b = q_first_bucket + t
                    for j in range(int(bucket_chunks[b])):
                        chunk_bucket.append((b, j == 0, j == int(bucket_chunks[b]) - 1))
                assert len(chunk_bucket) == q_nch

                acc_ps = None
                for blk0 in range(0, q_nch, GBLK):
                    nch = min(GBLK, q_nch - blk0)
                    for _j in range(nch):
                        _t = chunk_bucket[blk0 + _j][0] - q_first_bucket
                        if _t not in built:
                            build_tile(_t)
                    base_slot = q_start + blk0 * P

                    idx_t = idxpool.tile([P, GBLK * 8], mybir.dt.int16, tag="idx")
                    nc.sync.dma_start(
                        out=idx_t[:, :nch * 8],
                        in_=gidx_ext[:, base_slot // 16:(base_slot + nch * P) // 16])
                    slot_t = idxpool.tile([P, GBLK], bf16, tag="slot")
                    nc.sync.dma_start(
                        out=slot_t[:, :nch],
                        in_=slot_ext[:, base_slot // P:base_slot // P + nch])

                    g = gpool.tile([P, GBLK, d], f32, tag="g")
                    nc.gpsimd.dma_gather(
                        out_ap=g[:, :nch, :],
                        in_ap=feat_ext[q * chunk:(q + 1) * chunk, :],
                        idxs_ap=idx_t[:, :nch * 8],
                        num_idxs=nch * P,
                        num_idxs_reg=nch * P,
                        elem_size=d,
                        single_packet=False,
                    )

                    m_t = mpool.tile([P, GBLK, P], bf16, tag="m")
                    nc.vector.tensor_tensor(
                        out=m_t[:, :nch, :],
                        in0=slot_t[:, :nch, None].to_broadcast([P, nch, P]),
                        in1=iota_t[:, None, :].to_broadcast([P, nch, P]),
                        op=eq)

                    # M^T via TensorE transposes (groups of 4) + ACT evacuation,
                    # then per-edge dst rows via M @ T_scaled into half-block
                    # PSUM tiles (1 bank each).
                    HALF = 8
                    dps_halves = []
                    for g0 in range(0, nch, 4):
                        ng = min(4, nch - g0)
                        mtp = psA.tile([P, 4, P], bf16, tag="mtp")
                        for j in range(g0, g0 + ng):
                            nc.tensor.transpose(
                                mtp[:, j - g0, :], m_t[:, j, :], ident[:])
                        mts = mtspool.tile([P, 4, P], bf16, tag="mts")
                        nc.scalar.activation(
                            mts[:, :ng, :], mtp[:, :ng, :], AF.Copy)
                        if g0 % HALF == 0:
                            dps = psB.tile([P, HALF, d], f32, tag="dps")
                            dps_halves.append(dps)
                        for j in range(g0, g0 + ng):
                            cgl = blk0 + j
                            b, _, _ = chunk_bucket[cgl]
                            t = b - q_first_bucket
                            nc.tensor.matmul(
                                dps[:, j % HALF, :], lhsT=mts[:, j - g0, :],
                                rhs=tsc[:, t, :], start=True, stop=True)

                    S_blk = g[:, :nch, :]
                    ssq = prodpool.tile([P, GBLK, d], f32, tag="ssq")
                    nc.vector.tensor_tensor(out=ssq[:, :nch, :], in0=S_blk, in1=S_blk, op=mul)
                    ls2 = smpool.tile([P, GBLK], f32, tag="ls2")
                    nc.vector.tensor_reduce(
                        out=ls2[:, :nch], in_=ssq[:, :nch, :], axis=AX.X, op=add)
                    cosn = smpool.tile([P, GBLK], f32, tag="cosn")
                    for hi, dps in enumerate(dps_halves):
                        h0 = hi * HALF
                        nh = min(HALF, nch - h0)
                        sdp = prodpool.tile([P, HALF, d], f32, tag="sdp")
                        nc.vector.tensor_tensor(
                            out=sdp[:, :nh, :], in0=g[:, h0:h0 + nh, :],
                            in1=dps[:, :nh, :], op=mul)
                        nc.vector.tensor_reduce(
                            out=cosn[:, h0:h0 + nh], in_=sdp[:, :nh, :], axis=AX.X, op=add)
                    ls = smpool.tile([P, GBLK], f32, tag="ls")
                    nc.scalar.activation(ls[:, :nch], ls2[:, :nch], AF.Sqrt)
                    ils = smpool.tile([P, GBLK], f32, tag="ils")
                    nc.vector.reciprocal(ils[:, :nch], ls[:, :nch])
                    lg = smpool.tile([P, GBLK], f32, tag="lg")
                    nc.vector.tensor_tensor(
                        out=lg[:, :nch], in0=cosn[:, :nch], in1=ils[:, :nch], op=mul)
                    pt = smpool.tile([P, GBLK], f32, tag="pt")
                    nc.scalar.activation(
                        pt[:, :nch], lg[:, :nch], AF.Exp, scale=beta_t[:, 0:1])
                    xw = xwpool.tile([P, GBLK, d + 1], bf16, tag="xw")
                    nc.vector.tensor_tensor(
                        out=xw[:, :nch, 0:d], in0=S_blk,
                        in1=pt[:, :nch, None].to_broadcast([P, nch, d]), op=mul)
                    nc.vector.tensor_copy(out=xw[:, :nch, d], in_=pt[:, :nch])

                    # scatter matmuls
                    for j in range(nch):
                        cgl = blk0 + j
                        b, first, last = chunk_bucket[cgl]
                        t = b - q_first_bucket
                        if first:
                            acc_ps = psC.tile([P, d + 1], f32, tag="accps")
                        nc.tensor.matmul(
                            acc_ps[:], lhsT=m_t[:, j, :],
                            rhs=xw[:, j, :], start=first, stop=last)
                        if last:
                            nc.vector.tensor_tensor(
                                out=accum[:, t, :], in0=accum[:, t, :],
                                in1=acc_ps[:], op=add)

            for t in range(tiles):
                if t not in built:
                    build_tile(t)

            # ---- phase 3: normalize + writeback
            for t in range(tiles):
                r = smpool.tile([P, 1], f32, tag="r")
                nc.vector.reciprocal(r[:], accum[:, t, d:d + 1])
                ostg = tbpool.tile([P, d], f32, tag="ostg")
                nc.vector.tensor_scalar_mul(out=ostg[:], in0=accum[:, t, 0:d], scalar1=r[:])
                nc.sync.dma_start(out=out_ext[t * P:(t + 1) * P, :], in_=ostg[:])

    nc.compile()
    return nc


def _run(feat, beta, src, dst, cfg):
    global last_exec_ns
    n = cfg['n']
    n_pad = cfg['n_pad']
    d = cfg['d']
    n_cores = cfg['n_cores']
    nloc = cfg['nloc']
    tiles = cfg['tiles']
    chunk = cfg['chunk']
    nchunks = cfg['nchunks']
    nloc_pad = tiles * P

    feat = np.ascontiguousarray(np.asarray(feat, dtype=np.float32))
    beta = np.asarray(beta, dtype=np.float32)

    (bucket_chunks, bucket_off, s_total, gidx_streams, slot_streams,
     (node_core, node_tile, node_slot)) = _host_structure(
        src, dst, n, n_cores, nloc, tiles, chunk, nchunks)

    nc = _build_graph(cfg, bucket_chunks, bucket_off, s_total)

    feat_pad = np.zeros((n_pad, d), dtype=np.float32)
    feat_pad[:n] = feat
    beta128 = np.full((P, 1), beta.reshape(-1)[0], dtype=np.float32)
    import ml_dtypes
    iota128 = np.broadcast_to(np.arange(P).astype(ml_dtypes.bfloat16), (P, P)).copy()

    node_pos = node_tile * P + node_slot  # local row within the owning core
    in_maps = []
    for c in range(n_cores):
        locfeat = np.zeros((nloc_pad, d), dtype=np.float32)
        mine = np.nonzero(node_core == c)[0]
        locfeat[node_pos[mine]] = feat[mine]
        in_maps.append({
            "feat": feat_pad,
            "locfeat": locfeat,
            "beta128": beta128,
            "iota128": iota128,
            "gidx": gidx_streams[c],
            "slotw": slot_streams[c],
        })

    res = run_bass_kernel_spmd(nc, in_maps, core_ids=list(range(n_cores)),
                               trace=cfg.get('trace', False))
    last_exec_ns = res.exec_time_ns

    out = np.empty((n, d), dtype=np.float32)
    for c in range(n_cores):
        mine = np.nonzero(node_core == c)[0]
        out[mine] = res.results[c]["out"][node_pos[mine]]
    return out


FULL_CFG = dict(n=100000, n_pad=100352, d=64, n_cores=8, nloc=12500,
                tiles=104, chunk=25088, nchunks=4)


def kernel(feat, beta, src, dst):
    return _run(feat, beta, src, dst, dict(FULL_CFG))
